# revision 9
# baseline (speedup 1.0000x reference)
"""Trainium2 Bass kernel for 2-layer BiLSTM + classifier (nn_BiLSTM_45234595561814).

Strategy (8 NeuronCores, single SPMD launch, no collectives):
  - Each core q owns a 64-token window W_q = [64q, 64q+64) of T=512, FULL batch
    (B=64).  The window is split into S=2 sub-windows of 32 tokens; each
    sub-window runs BOTH directions -> 4 independent interleaved chains per
    layer phase.  The LSTM recurrence is serial per chain (~2us per-step
    cross-engine latency), so wall time ~ iterations x step-latency; more
    chains per iteration cuts iterations while engines stay below saturation.
  - Sequence parallelism via truncated warmup: LSTM state decays ~0.5/step, so
    a chain zero-initialized WARM steps before its sub-window converges to the
    exact state (err ~ WARM * 2^-WARM).  Layer-0 chains span
    [sub-window -W, +SW+W) so layer-1 warmups are fed locally -> the (L0,L1)
    cascade self-warms; no cross-core exchange anywhere.
  - One-tanh trick: i,f,o weight rows pre-scaled by 0.5 so sigmoid(z) =
    0.5*(1+tanh(z/2)) needs only tanh -> ONE ACT op for all 4 gates
    ([o|i|f|g] in one PSUM region).  State kept doubled (C=2c, hh=2h); cell
    update is 3 DVE scalar_tensor_tensor ops; Whh pre-scaled by extra 0.5 to
    absorb hh=2h.
  - Both directions of a sub-window share one PSUM tile (f cols 0:4B, b cols
    4B:8B) so PSUM fits in banks; subtile dependency tracking keeps the two
    chains' schedules independent.
  - L0 input projection fused into the per-step PSUM accumulation (K=65 with
    a ones row carrying the bias).  L1 projection computed ON THE FLY from the
    SBUF-resident y0 outputs (12 small matmuls per step: 4 gates x
    {y0f-K-tile, y0b-K-tile, ctl}) -- no DRAM round-trip, no scatter copies.
  - Pad tokens (outside [0,512)) handled exactly: x/ones rows zero keep L0
    state at 0 through leading pads; an L1 control row drives the i-gate
    preact to -30000 on pad tokens so pad xg1 cannot perturb state.
  - Classifier is fully local; final GEMM emitted transposed (tokens on
    partitions) so the output DMA is contiguous; tanh batched 4 chunks/op.

kernel(**inputs) takes the FULL inputs and returns the FULL [64,512,64] f32
output.  Self-contained: hardcodes all shapes; no sibling imports.
"""

import os

import numpy as np
import ml_dtypes

import concourse.bass as bass
import concourse.mybir as mybir
import concourse.tile as tile
from concourse import bacc
from concourse.bass_utils import run_bass_kernel_spmd

bf16 = ml_dtypes.bfloat16
F32, BF16 = mybir.dt.float32, mybir.dt.bfloat16
AluOp = mybir.AluOpType
ACT_TANH = mybir.ActivationFunctionType.Tanh
ACT_RELU = mybir.ActivationFunctionType.Relu

H = 128          # rnn size
B = 64           # batch
T = 512          # seq len
D = 64           # input size
NC = 8           # cores
WIN = T // NC    # tokens per core window = 64
S = 2            # sub-windows per core
SW = WIN // S    # tokens per sub-window = 32
WARM = int(os.environ.get("BILSTM_WARM", "12"))
SPAN0 = SW + 2 * WARM    # L0 chain steps per sub-window chain
SPAN1 = SW + WARM        # L1 chain steps
SPANX = WIN + 2 * WARM   # xaug slots per core
PADKILL = -30000.0

CHAINS = [(w, d) for w in range(S) for d in "fb"]

_CACHE = {}


def _build_program():
    nc = bacc.Bacc(None, target_bir_lowering=False)

    # ---------------- I/O declarations ----------------
    ei = lambda name, shape, dt=BF16: nc.dram_tensor(name, shape, dt, kind="ExternalInput")
    xaug = ei("xaug", [D + 1, SPANX * B])          # rows 0..63 x.T, row 64 valid-ones
    ctl1 = ei("ctl1", [2, SPANX * B])              # row0 valid, row1 padkill indicator
    wihT0 = {d: ei(f"wihT0{d}", [D + 1, 4 * H]) for d in "fb"}
    whhT0 = {d: ei(f"whhT0{d}", [H, 4 * H]) for d in "fb"}
    whhT1 = {d: ei(f"whhT1{d}", [H, 4 * H]) for d in "fb"}
    wih1Ta = {d: ei(f"wih1Ta{d}", [H, 4 * H]) for d in "fb"}   # y0f K-tile
    wih1Tb = {d: ei(f"wih1Tb{d}", [H, 4 * H]) for d in "fb"}   # y0b K-tile
    ctlT1 = {d: ei(f"ctlT1{d}", [2, 4 * H]) for d in "fb"}     # bias row + padkill row
    w1Ta = ei("w1Ta", [H, 2 * H])   # (0.5*W1).T rows 0:128  -> [128, 256]
    w1Tb = ei("w1Tb", [H, 2 * H])   # rows 128:256
    b1row = ei("b1row", [1, 2 * H])
    w2Ta = ei("w2Ta", [H, D])       # W2.T rows 0:128 -> [128, 64]
    w2Tb = ei("w2Tb", [H, D])
    b2row = ei("b2row", [1, D])
    out = nc.dram_tensor("out", [WIN * B, D], F32, kind="ExternalOutput")

    with tile.TileContext(nc) as tc:
        with tc.tile_pool(name="singles", bufs=1) as singles, \
             tc.tile_pool(name="state", bufs=1) as state, \
             tc.tile_pool(name="tpool", bufs=4) as tpool, \
             tc.tile_pool(name="vpool", bufs=3) as vpool, \
             tc.tile_pool(name="clssb", bufs=3) as clssb, \
             tc.tile_pool(name="psA", bufs=3, space="PSUM") as psA, \
             tc.tile_pool(name="psB", bufs=3, space="PSUM") as psB, \
             tc.tile_pool(name="psP", bufs=2, space="PSUM") as psP:

            gpool = {0: psA, 1: psB}

            # ---------------- load constants ----------------
            def load(src, shape, dt=BF16):
                t = singles.tile(shape, dt, name=src.name, tag=src.name)
                nc.sync.dma_start(out=t[:], in_=src[:])
                return t

            xaug_t = load(xaug, [D + 1, SPANX * B])
            ctl1_t = load(ctl1, [2, SPANX * B])
            wihT0_t = {d: load(wihT0[d], [D + 1, 4 * H]) for d in "fb"}
            whhT0_t = {d: load(whhT0[d], [H, 4 * H]) for d in "fb"}
            whhT1_t = {d: load(whhT1[d], [H, 4 * H]) for d in "fb"}
            wih1Ta_t = {d: load(wih1Ta[d], [H, 4 * H]) for d in "fb"}
            wih1Tb_t = {d: load(wih1Tb[d], [H, 4 * H]) for d in "fb"}
            ctlT1_t = {d: load(ctlT1[d], [2, 4 * H]) for d in "fb"}
            w1Ta_t = load(w1Ta, [H, 2 * H])
            w1Tb_t = load(w1Tb, [H, 2 * H])
            b1row_t = load(b1row, [1, 2 * H])
            w2Ta_t = load(w2Ta, [H, D])
            w2Tb_t = load(w2Tb, [H, D])
            b2row_t = load(b2row, [1, D])

            # ---------------- persistent state ----------------
            # y0/y1 indexed by SPATIAL slot (b chains write descending).
            y0 = {c: state.tile([H, SPAN0 * B], BF16, name=f"y0{c[0]}{c[1]}", tag=f"y0{c[0]}{c[1]}")
                  for c in CHAINS}
            y1 = {c: state.tile([H, SPAN1 * B], BF16, name=f"y1{c[0]}{c[1]}", tag=f"y1{c[0]}{c[1]}")
                  for c in CHAINS}
            h00 = state.tile([H, B], BF16, name="h00", tag="h00")
            nc.vector.memset(h00[:], 0.0)

            # xaug slot of a chain's spatial slot 0 (sub-window w's L0 span
            # starts at xaug slot SW*w).
            xoff0 = {w: SW * w for w in range(S)}

            # ---------------- generic LSTM machinery ----------------
            # Gate col order in the psum pair-tile: chain f cols [0:4B),
            # chain b cols [4B:8B); within a chain [o | i | f | g]*B.
            # Cell state C=2c lives in t-tile cols 4B:5B (written by the
            # PREVIOUS step's c-update into THIS step's tile, so (1+ti)*tg and
            # (1+tf)*C fuse into one scalar_tensor_tensor over [i|f] x [g|C]).
            # start_tensor_calc marks the WHOLE 2KB PSUM bank (zero region)
            # pending-zero, so exactly ONE matmul per pair-tile generation may
            # carry start=True: chain f's first inproj matmul.  Chain b's first
            # writes then land on pending bytes and overwrite correctly.
            def pair_prefetch(w, inproj, first=False):
                g_pair = gpool[w].tile([H, 8 * B], F32, name=f"g{w}", tag=f"g{w}")
                if inproj is not None:
                    inproj((w, "f"), g_pair[:, 0:4 * B], True)
                    inproj((w, "b"), g_pair[:, 4 * B:8 * B], False)
                tt = {}
                for d in "fb":
                    tt[d] = tpool.tile([H, 5 * B], F32, name=f"t{w}{d}", tag=f"t{w}{d}")
                    if first:
                        nc.vector.memset(tt[d][:, 4 * B:5 * B], 0.0)
                return g_pair, tt

            def gslice(cur, c):
                off = 0 if c[1] == "f" else 4 * B
                return cur[0][:, off:off + 4 * B]

            def run_phase(span, inproj, whh, hprev_fn, yout_fn):
                pend = {}
                for step in range(span + 1):
                    for w in range(S):
                        ip = None
                        if step < span:
                            ip = lambda c, ps, lead, ss=step: inproj(c, ss, ps, lead)
                        pend[(w, step)] = pair_prefetch(w, ip, first=(step == 0))
                    if step < 1:
                        continue
                    p = step - 1
                    cur = {w: pend.pop((w, p)) for w in range(S)}
                    nxt = {w: pend[(w, step)] for w in range(S)}
                    for c in CHAINS:           # recurrent matmuls
                        g = gslice(cur[c[0]], c)
                        hp = hprev_fn(c, p)
                        for gi in range(4):
                            nc.tensor.matmul(g[:, gi * B:(gi + 1) * B],
                                             whh[c][:, gi * H:(gi + 1) * H], hp,
                                             start=False, stop=True,
                                             skip_group_check=True)
                    for c in CHAINS:           # one ACT: all 4 gates tanh
                        nc.scalar.activation(cur[c[0]][1][c[1]][:, 0:4 * B],
                                             gslice(cur[c[0]], c), ACT_TANH)
                    scr = {}
                    for c in CHAINS:           # scr = [(1+ti)*tg | (1+tf)*C]
                        t_t = cur[c[0]][1][c[1]]
                        sc = vpool.tile([H, 2 * B], F32, name=f"s{c[0]}{c[1]}",
                                        tag=f"s{c[0]}{c[1]}")
                        nc.vector.scalar_tensor_tensor(sc[:], t_t[:, B:3 * B], 1.0,
                                                       t_t[:, 3 * B:5 * B],
                                                       AluOp.add, AluOp.mult)
                        scr[c] = sc
                    for c in CHAINS:           # C' = 0.5*(1+tf)*C + (1+ti)*tg
                        Tn = nxt[c[0]][1][c[1]]
                        nc.vector.scalar_tensor_tensor(Tn[:, 4 * B:5 * B],
                                                       scr[c][:, B:2 * B], 0.5,
                                                       scr[c][:, 0:B],
                                                       AluOp.mult, AluOp.add)
                    tcs = {}
                    for c in CHAINS:           # tc = tanh(C'/2)
                        Tn = nxt[c[0]][1][c[1]]
                        tc_t = vpool.tile([H, B], F32, name=f"c{c[0]}{c[1]}",
                                          tag=f"c{c[0]}{c[1]}")
                        nc.scalar.activation(tc_t[:], Tn[:, 4 * B:5 * B],
                                             ACT_TANH, scale=0.5)
                        tcs[c] = tc_t
                    for c in CHAINS:           # h = (1+to)*tc   (doubled h)
                        t_t = cur[c[0]][1][c[1]]
                        nc.vector.scalar_tensor_tensor(yout_fn(c, p), t_t[:, 0:B],
                                                       1.0, tcs[c][:],
                                                       AluOp.add, AluOp.mult)

            # ---------------- layer 0 ----------------
            # chain step p -> spatial slot: f ascending, b descending.
            def sp0(c, p):
                return p if c[1] == "f" else SPAN0 - 1 - p

            def l0_inproj(c, s, g_ps, lead):
                sl = xoff0[c[0]] + sp0(c, s)
                for gi in range(4):
                    nc.tensor.matmul(g_ps[:, gi * B:(gi + 1) * B],
                                     wihT0_t[c[1]][:, gi * H:(gi + 1) * H],
                                     xaug_t[:, sl * B:(sl + 1) * B],
                                     start=(gi == 0 and lead), stop=False,
                                     skip_group_check=True)

            def l0_hprev(c, p):
                if p == 0:
                    return h00[:]
                sl = sp0(c, p - 1)
                return y0[c][:, sl * B:(sl + 1) * B]

            def l0_yout(c, p):
                sl = sp0(c, p)
                return y0[c][:, sl * B:(sl + 1) * B]

            run_phase(SPAN0, l0_inproj, {c: whhT0_t[c[1]] for c in CHAINS},
                      l0_hprev, l0_yout)

            # ---------------- layer 1 (on-the-fly input projection) ----------------
            # L1-f covers spatial slots [0, SPAN1); L1-b covers [WARM, SPAN0).
            # y1 stored y1-locally: f local slot = spatial; b local = spatial-WARM.
            def sp1(c, p):
                return p if c[1] == "f" else SPAN0 - 1 - p    # spatial (L0 coords)

            def l1_inproj(c, s, g_ps, lead):
                ys = sp1(c, s)
                xs = xoff0[c[0]] + ys
                yf = y0[(c[0], "f")][:, ys * B:(ys + 1) * B]
                yb = y0[(c[0], "b")][:, ys * B:(ys + 1) * B]
                ct = ctl1_t[:, xs * B:(xs + 1) * B]
                d = c[1]
                for gi in range(4):
                    nc.tensor.matmul(g_ps[:, gi * B:(gi + 1) * B],
                                     wih1Ta_t[d][:, gi * H:(gi + 1) * H], yf,
                                     start=(gi == 0 and lead), stop=False,
                                     skip_group_check=True)
                    nc.tensor.matmul(g_ps[:, gi * B:(gi + 1) * B],
                                     wih1Tb_t[d][:, gi * H:(gi + 1) * H], yb,
                                     start=False, stop=False,
                                     skip_group_check=True)
                    nc.tensor.matmul(g_ps[:, gi * B:(gi + 1) * B],
                                     ctlT1_t[d][:, gi * H:(gi + 1) * H], ct,
                                     start=False, stop=False,
                                     skip_group_check=True)

            def y1loc(c, p):
                sl = sp1(c, p)
                return sl if c[1] == "f" else sl - WARM

            def l1_hprev(c, p):
                if p == 0:
                    return h00[:]
                sl = y1loc(c, p - 1)
                return y1[c][:, sl * B:(sl + 1) * B]

            def l1_yout(c, p):
                sl = y1loc(c, p)
                return y1[c][:, sl * B:(sl + 1) * B]

            run_phase(SPAN1, l1_inproj, {c: whhT1_t[c[1]] for c in CHAINS},
                      l1_hprev, l1_yout)

            # ---------------- classifier (window slots only) ----------------
            # window token u of sub-window w: spatial slot WARM+u ->
            #   y1f local slot WARM+u; y1b local slot u.  Both contiguous.
            NTOK = SW * B                       # tokens*batch per sub-window = 2048
            CH = 512
            h1 = {}
            for w in range(S):
                cf, cb = (w, "f"), (w, "b")
                h1[w] = [clssb.tile([H, NTOK], BF16, name=f"h1a{w}", tag=f"h1a{w}", bufs=1),
                         clssb.tile([H, NTOK], BF16, name=f"h1b{w}", tag=f"h1b{w}", bufs=1)]
                for c0 in range(0, NTOK, CH):
                    xs0 = (WARM + SW * w) * B + c0   # xaug col of token chunk
                    for m in range(2):
                        p = psP.tile([H, CH], F32, name="pc", tag="pp")
                        nc.tensor.matmul(p[:], w1Ta_t[:, m * H:(m + 1) * H],
                                         y1[cf][:, WARM * B + c0:WARM * B + c0 + CH],
                                         start=True, stop=False)
                        nc.tensor.matmul(p[:], w1Tb_t[:, m * H:(m + 1) * H],
                                         y1[cb][:, c0:c0 + CH],
                                         start=False, stop=False)
                        nc.tensor.matmul(p[:], b1row_t[:, m * H:(m + 1) * H],
                                         ctl1_t[0:1, xs0:xs0 + CH],
                                         start=False, stop=True)
                        nc.scalar.activation(h1[w][m][:, c0:c0 + CH], p[:], ACT_RELU)

            # final GEMM transposed: out[tok, d] (tokens on partitions);
            # batch tanh over 4 chunks of 128 token-rows (one PSUM tile).
            for w in range(S):
                for blk in range(0, NTOK, 4 * H):
                    p = psP.tile([H, 4 * D], F32, name="po", tag="pp")
                    for k in range(4):
                        c0 = blk + k * H
                        nc.tensor.matmul(p[:, k * D:(k + 1) * D],
                                         h1[w][0][:, c0:c0 + H], w2Ta_t[:],
                                         start=(k == 0), stop=False,
                                         skip_group_check=True)
                        nc.tensor.matmul(p[:, k * D:(k + 1) * D],
                                         h1[w][1][:, c0:c0 + H], w2Tb_t[:],
                                         start=False, stop=False,
                                         skip_group_check=True)
                        nc.tensor.matmul(p[:, k * D:(k + 1) * D],
                                         ctl1_t[0:1, (WARM + SW * w) * B + c0:(WARM + SW * w) * B + c0 + H],
                                         b2row_t[:],
                                         start=False, stop=(k == 3),
                                         skip_group_check=True)
                    o_t = clssb.tile([H, 4 * D], F32, name="ot", tag="ot")
                    nc.scalar.activation(o_t[:], p[:], ACT_TANH)
                    nc.sync.dma_start(
                        out=out[w * NTOK + blk:w * NTOK + blk + 4 * H, :].rearrange(
                            "(k p) d -> p k d", k=4),
                        in_=o_t[:].rearrange("p (k d) -> p k d", k=4))

    nc.compile()
    return nc


# ======================= host side =======================

def _prep_weights(inp):
    """Returns dict of np arrays shared by all cores (bf16).

    Gate row-blocks reordered from reference [i,f,g,o] to device [o,i,f,g];
    i,f,o rows scaled 0.5 (one-tanh trick)."""
    H_ = H
    sr = np.full((4 * H_, 1), 0.5, np.float32)
    sr[2 * H_:3 * H_] = 1.0

    def reorder(a):           # rows [i,f,g,o] -> [o,i,f,g]
        return np.concatenate([a[3 * H_:], a[:H_], a[H_:2 * H_], a[2 * H_:3 * H_]], 0)

    w = {}
    for d, tag in (("f", "0"), ("b", "1")):
        Wih, Whh = inp[f"Wih0{tag}"], inp[f"Whh0{tag}"]
        bias = inp[f"bih0{tag}"] + inp[f"bhh0{tag}"]
        w[f"wihT0{d}"] = reorder(np.concatenate([Wih * sr, (bias[:, None] * sr)], 1)).T.astype(bf16)
        w[f"whhT0{d}"] = reorder(Whh * sr * 0.5).T.astype(bf16)
        Wih1, Whh1 = inp[f"Wih1{tag}"], inp[f"Whh1{tag}"]
        bias1 = reorder((inp[f"bih1{tag}"] + inp[f"bhh1{tag}"])[:, None] * sr).T
        w[f"whhT1{d}"] = reorder(Whh1 * sr * 0.5).T.astype(bf16)
        w[f"wih1Ta{d}"] = reorder(Wih1[:, :H] * sr * 0.5).T.astype(bf16)
        w[f"wih1Tb{d}"] = reorder(Wih1[:, H:] * sr * 0.5).T.astype(bf16)
        padkill = np.zeros((1, 4 * H), np.float32)
        padkill[0, H:2 * H] = PADKILL      # i-gate block (device order [o,i,f,g])
        w[f"ctlT1{d}"] = np.concatenate([bias1, padkill], 0).astype(bf16)
    w["w1Ta"] = (0.5 * inp["W1"][:, :H]).T.astype(bf16)
    w["w1Tb"] = (0.5 * inp["W1"][:, H:]).T.astype(bf16)
    w["b1row"] = inp["b1"][None, :].astype(bf16)
    w["w2Ta"] = inp["W2"][:, :H].T.astype(bf16)
    w["w2Tb"] = inp["W2"][:, H:].T.astype(bf16)
    w["b2row"] = inp["b2"][None, :].astype(bf16)
    return w


def _per_core_inputs(x, q):
    """x: [B, T, D] f32.  Builds xaug [65, SPANX*B] and ctl1 [2, SPANX*B]."""
    t0 = WIN * q - WARM
    xaug = np.zeros((D + 1, SPANX * B), np.float32)
    ctl = np.zeros((2, SPANX * B), np.float32)
    for s in range(SPANX):
        t = t0 + s
        sl = slice(s * B, (s + 1) * B)
        if 0 <= t < T:
            xaug[:D, sl] = x[:, t, :].T
            xaug[D, sl] = 1.0
            ctl[0, sl] = 1.0
        else:
            ctl[1, sl] = 1.0
    return xaug.astype(bf16), ctl.astype(bf16)


def _get_program():
    if "nc" not in _CACHE:
        _CACHE["nc"] = _build_program()
    return _CACHE["nc"]


def _run(inputs, trace=False):
    inp = {k: np.asarray(v) for k, v in inputs.items()}
    nc = _get_program()
    w = _prep_weights(inp)
    x = inp["x"].astype(np.float32)
    in_maps = []
    for q in range(NC):
        xaug, ctl = _per_core_inputs(x, q)
        m = dict(w)
        m["xaug"] = xaug
        m["ctl1"] = ctl
        in_maps.append(m)
    res = run_bass_kernel_spmd(nc, in_maps, list(range(NC)), trace=trace)
    outp = np.zeros((B, T, D), np.float32)
    for q in range(NC):
        o = res.results[q]["out"].reshape(WIN, B, D)        # [tok, b, d]
        outp[:, WIN * q:WIN * (q + 1), :] = o.transpose(1, 0, 2)
    return outp, res


def kernel(**inputs):
    out, _ = _run(inputs, trace=False)
    return out


# revision 19
# speedup vs baseline: 1.5702x; 1.5702x over previous
"""Trainium2 Bass kernel for 2-layer BiLSTM + classifier (nn_BiLSTM_45234595561814).

Strategy (8 NeuronCores, single SPMD launch, no collectives):
  - Each core q owns a 64-token window W_q = [64q, 64q+64) of T=512, FULL batch
    (B=64).  The window is split into S=2 sub-windows of 32 tokens; each
    sub-window runs BOTH directions -> 4 independent interleaved chains per
    layer phase.  The LSTM recurrence is serial per chain (~2us per-step
    cross-engine latency), so wall time ~ iterations x step-latency; more
    chains per iteration cuts iterations while engines stay below saturation.
  - Sequence parallelism via truncated warmup: LSTM state decays ~0.5/step, so
    a chain zero-initialized WARM steps before its sub-window converges to the
    exact state (err ~ WARM * 2^-WARM).  Layer-0 chains span
    [sub-window -W, +SW+W) so layer-1 warmups are fed locally -> the (L0,L1)
    cascade self-warms; no cross-core exchange anywhere.
  - One-tanh trick: i,f,o weight rows pre-scaled by 0.5 so sigmoid(z) =
    0.5*(1+tanh(z/2)) needs only tanh -> ONE ACT op for all 4 gates
    ([o|i|f|g] in one PSUM region).  State kept doubled (C=2c, hh=2h); cell
    update is 3 DVE scalar_tensor_tensor ops; Whh pre-scaled by extra 0.5 to
    absorb hh=2h.
  - Both directions of a sub-window share one PSUM tile (f cols 0:4B, b cols
    4B:8B) so PSUM fits in banks; subtile dependency tracking keeps the two
    chains' schedules independent.
  - L0 input projection fused into the per-step PSUM accumulation (K=65 with
    a ones row carrying the bias).  L1 projection computed ON THE FLY from the
    SBUF-resident y0 outputs (12 small matmuls per step: 4 gates x
    {y0f-K-tile, y0b-K-tile, ctl}) -- no DRAM round-trip, no scatter copies.
  - Pad tokens (outside [0,512)) handled exactly: x/ones rows zero keep L0
    state at 0 through leading pads; an L1 control row drives the i-gate
    preact to -30000 on pad tokens so pad xg1 cannot perturb state.
  - Classifier is fully local; final GEMM emitted transposed (tokens on
    partitions) so the output DMA is contiguous; tanh batched 4 chunks/op.

kernel(**inputs) takes the FULL inputs and returns the FULL [64,512,64] f32
output.  Self-contained: hardcodes all shapes; no sibling imports.
"""

import os

import numpy as np
import ml_dtypes

import concourse.bass as bass
import concourse.mybir as mybir
import concourse.tile as tile
from concourse import bacc
from concourse.bass_utils import run_bass_kernel_spmd

bf16 = ml_dtypes.bfloat16
F32, BF16 = mybir.dt.float32, mybir.dt.bfloat16
AluOp = mybir.AluOpType
ACT_TANH = mybir.ActivationFunctionType.Tanh
ACT_RELU = mybir.ActivationFunctionType.Relu

H = 128          # rnn size
B = 64           # batch
T = 512          # seq len
D = 64           # input size
NC = 8           # cores
WIN = T // NC    # tokens per core window = 64
S = 2            # sub-windows per core
SW = WIN // S    # tokens per sub-window = 32
WARM = int(os.environ.get("BILSTM_WARM", "12"))
SPAN0 = SW + 2 * WARM    # L0 chain steps per sub-window chain
SPAN1 = SW + WARM        # L1 chain steps
SPANX = WIN + 2 * WARM   # xaug slots per core
PADKILL = -30000.0

CHAINS = [(w, d) for w in range(S) for d in "fb"]

_CACHE = {}


def _build_program():
    nc = bacc.Bacc(None, target_bir_lowering=False)

    # ---------------- I/O declarations ----------------
    # xaug rows: 0..D-1 = x.T, D = valid-ones, D+1 = pad indicator.  All
    # "control" matmuls (L1 bias+padkill, classifier biases) use the full
    # 66-row xaug as one operand so K>64 everywhere -> no PE tile-config
    # switches (K<=64 matmuls force a ~170ns row-group reconfig each).
    KA = D + 2
    ei = lambda name, shape, dt=BF16: nc.dram_tensor(name, shape, dt, kind="ExternalInput")
    xaug = ei("xaug", [KA, SPANX * B])
    wihT0 = {d: ei(f"wihT0{d}", [D + 1, 4 * H]) for d in "fb"}
    whhT0 = {d: ei(f"whhT0{d}", [H, 4 * H]) for d in "fb"}
    whhT1 = {d: ei(f"whhT1{d}", [H, 4 * H]) for d in "fb"}
    wih1Ta = {d: ei(f"wih1Ta{d}", [H, 4 * H]) for d in "fb"}   # y0f K-tile
    wih1Tb = {d: ei(f"wih1Tb{d}", [H, 4 * H]) for d in "fb"}   # y0b K-tile
    ctlT66 = {d: ei(f"ctlT66{d}", [KA, 4 * H]) for d in "fb"}  # rows D/D+1: bias/padkill
    w1Ta = ei("w1Ta", [H, 2 * H])   # (0.5*W1).T rows 0:128  -> [128, 256]
    w1Tb = ei("w1Tb", [H, 2 * H])   # rows 128:256
    b1rowP = ei("b1rowP", [KA, 2 * H])   # zeros; row D = b1
    w2Ta = ei("w2Ta", [H, D])       # W2.T rows 0:128 -> [128, 64]
    w2Tb = ei("w2Tb", [H, D])
    b2rowP = ei("b2rowP", [KA, D])       # zeros; row D = b2
    out = nc.dram_tensor("out", [WIN * B, D], F32, kind="ExternalOutput")

    with tile.TileContext(nc) as tc:
        with tc.tile_pool(name="singles", bufs=1) as singles, \
             tc.tile_pool(name="state", bufs=1) as state, \
             tc.tile_pool(name="tpool", bufs=4) as tpool, \
             tc.tile_pool(name="vpool", bufs=3) as vpool, \
             tc.tile_pool(name="clssb", bufs=3) as clssb, \
             tc.tile_pool(name="psA", bufs=4, space="PSUM") as psA, \
             tc.tile_pool(name="psB", bufs=4, space="PSUM") as psB:

            gpool = {0: psA, 1: psB}

            # ---------------- load constants ----------------
            def load(src, shape, dt=BF16):
                t = singles.tile(shape, dt, name=src.name, tag=src.name)
                nc.sync.dma_start(out=t[:], in_=src[:])
                return t

            xaug_t = load(xaug, [KA, SPANX * B])
            wihT0_t = {d: load(wihT0[d], [D + 1, 4 * H]) for d in "fb"}
            whhT0_t = {d: load(whhT0[d], [H, 4 * H]) for d in "fb"}
            whhT1_t = {d: load(whhT1[d], [H, 4 * H]) for d in "fb"}
            wih1Ta_t = {d: load(wih1Ta[d], [H, 4 * H]) for d in "fb"}
            wih1Tb_t = {d: load(wih1Tb[d], [H, 4 * H]) for d in "fb"}
            ctlT66_t = {d: load(ctlT66[d], [KA, 4 * H]) for d in "fb"}
            w1Ta_t = load(w1Ta, [H, 2 * H])
            w1Tb_t = load(w1Tb, [H, 2 * H])
            b1rowP_t = load(b1rowP, [KA, 2 * H])
            w2Ta_t = load(w2Ta, [H, D])
            w2Tb_t = load(w2Tb, [H, D])
            b2rowP_t = load(b2rowP, [KA, D])

            # ---------------- persistent state ----------------
            # y0/y1 indexed by SPATIAL slot (b chains write descending).
            y0 = {c: state.tile([H, SPAN0 * B], BF16, name=f"y0{c[0]}{c[1]}", tag=f"y0{c[0]}{c[1]}")
                  for c in CHAINS}
            y1 = {c: state.tile([H, SPAN1 * B], BF16, name=f"y1{c[0]}{c[1]}", tag=f"y1{c[0]}{c[1]}")
                  for c in CHAINS}
            h00 = state.tile([H, B], BF16, name="h00", tag="h00")
            nc.vector.memset(h00[:], 0.0)

            # xaug slot of a chain's spatial slot 0 (sub-window w's L0 span
            # starts at xaug slot SW*w).
            xoff0 = {w: SW * w for w in range(S)}

            # ---------------- generic LSTM machinery ----------------
            # Gate col order in the psum pair-tile: chain f cols [0:4B),
            # chain b cols [4B:8B); within a chain [o | i | f | g]*B.
            # Cell state C=2c lives in t-tile cols 4B:5B (written by the
            # PREVIOUS step's c-update into THIS step's tile, so (1+ti)*tg and
            # (1+tf)*C fuse into one scalar_tensor_tensor over [i|f] x [g|C]).
            # start_tensor_calc marks the WHOLE 2KB PSUM bank (zero region)
            # pending-zero, so exactly ONE matmul per pair-tile generation may
            # carry start=True: chain f's first inproj matmul.  Chain b's first
            # writes then land on pending bytes and overwrite correctly.
            def pair_prefetch(w, inproj, first=False):
                g_pair = gpool[w].tile([H, 8 * B], F32, name=f"g{w}", tag=f"g{w}")
                if inproj is not None:
                    inproj((w, "f"), g_pair[:, 0:4 * B], True)
                    inproj((w, "b"), g_pair[:, 4 * B:8 * B], False)
                tt = {}
                for d in "fb":
                    tt[d] = tpool.tile([H, 5 * B], F32, name=f"t{w}{d}", tag=f"t{w}{d}")
                    if first:
                        nc.vector.memset(tt[d][:, 4 * B:5 * B], 0.0)
                return g_pair, tt

            def gslice(cur, c):
                off = 0 if c[1] == "f" else 4 * B
                return cur[0][:, off:off + 4 * B]

            def run_phase(span, inproj, whh, hprev_fn, yout_fn):
                pend = {}
                for step in range(span + 1):
                    for w in range(S):
                        ip = None
                        if step < span:
                            ip = lambda c, ps, lead, ss=step: inproj(c, ss, ps, lead)
                        pend[(w, step)] = pair_prefetch(w, ip, first=(step == 0))
                    if step < 1:
                        continue
                    p = step - 1
                    cur = {w: pend.pop((w, p)) for w in range(S)}
                    nxt = {w: pend[(w, step)] for w in range(S)}
                    for c in CHAINS:           # recurrent matmuls
                        g = gslice(cur[c[0]], c)
                        hp = hprev_fn(c, p)
                        for gi in range(4):
                            nc.tensor.matmul(g[:, gi * B:(gi + 1) * B],
                                             whh[c][:, gi * H:(gi + 1) * H], hp,
                                             start=False, stop=True,
                                             skip_group_check=True)
                    for c in CHAINS:           # one ACT: all 4 gates tanh
                        nc.scalar.activation(cur[c[0]][1][c[1]][:, 0:4 * B],
                                             gslice(cur[c[0]], c), ACT_TANH)
                    scr = {}
                    for c in CHAINS:           # scr = [(1+ti)*tg | (1+tf)*C]
                        t_t = cur[c[0]][1][c[1]]
                        sc = vpool.tile([H, 2 * B], F32, name=f"s{c[0]}{c[1]}",
                                        tag=f"s{c[0]}{c[1]}")
                        nc.vector.scalar_tensor_tensor(sc[:], t_t[:, B:3 * B], 1.0,
                                                       t_t[:, 3 * B:5 * B],
                                                       AluOp.add, AluOp.mult)
                        scr[c] = sc
                    for c in CHAINS:           # C' = 0.5*(1+tf)*C + (1+ti)*tg
                        Tn = nxt[c[0]][1][c[1]]
                        nc.vector.scalar_tensor_tensor(Tn[:, 4 * B:5 * B],
                                                       scr[c][:, B:2 * B], 0.5,
                                                       scr[c][:, 0:B],
                                                       AluOp.mult, AluOp.add)
                    tcs = {}
                    for c in CHAINS:           # tc = tanh(C'/2)
                        Tn = nxt[c[0]][1][c[1]]
                        tc_t = vpool.tile([H, B], F32, name=f"c{c[0]}{c[1]}",
                                          tag=f"c{c[0]}{c[1]}")
                        nc.scalar.activation(tc_t[:], Tn[:, 4 * B:5 * B],
                                             ACT_TANH, scale=0.5)
                        tcs[c] = tc_t
                    for c in CHAINS:           # h = (1+to)*tc   (doubled h)
                        t_t = cur[c[0]][1][c[1]]
                        nc.vector.scalar_tensor_tensor(yout_fn(c, p), t_t[:, 0:B],
                                                       1.0, tcs[c][:],
                                                       AluOp.add, AluOp.mult)

            # ---------------- layer 0 ----------------
            # chain step p -> spatial slot: f ascending, b descending.
            def sp0(c, p):
                return p if c[1] == "f" else SPAN0 - 1 - p

            def l0_inproj(c, s, g_ps, lead):
                sl = xoff0[c[0]] + sp0(c, s)
                for gi in range(4):
                    nc.tensor.matmul(g_ps[:, gi * B:(gi + 1) * B],
                                     wihT0_t[c[1]][:, gi * H:(gi + 1) * H],
                                     xaug_t[0:D + 1, sl * B:(sl + 1) * B],
                                     start=(gi == 0 and lead), stop=False,
                                     skip_group_check=True)

            def l0_hprev(c, p):
                if p == 0:
                    return h00[:]
                sl = sp0(c, p - 1)
                return y0[c][:, sl * B:(sl + 1) * B]

            def l0_yout(c, p):
                sl = sp0(c, p)
                return y0[c][:, sl * B:(sl + 1) * B]

            run_phase(SPAN0, l0_inproj, {c: whhT0_t[c[1]] for c in CHAINS},
                      l0_hprev, l0_yout)

            # ---------------- layer 1 (on-the-fly input projection) ----------------
            # L1-f covers spatial slots [0, SPAN1); L1-b covers [WARM, SPAN0).
            # y1 stored y1-locally: f local slot = spatial; b local = spatial-WARM.
            def sp1(c, p):
                return p if c[1] == "f" else SPAN0 - 1 - p    # spatial (L0 coords)

            def l1_inproj(c, s, g_ps, lead):
                ys = sp1(c, s)
                xs = xoff0[c[0]] + ys
                yf = y0[(c[0], "f")][:, ys * B:(ys + 1) * B]
                yb = y0[(c[0], "b")][:, ys * B:(ys + 1) * B]
                ct = xaug_t[:, xs * B:(xs + 1) * B]   # rows D/D+1 select bias/padkill
                d = c[1]
                for gi in range(4):
                    nc.tensor.matmul(g_ps[:, gi * B:(gi + 1) * B],
                                     wih1Ta_t[d][:, gi * H:(gi + 1) * H], yf,
                                     start=(gi == 0 and lead), stop=False,
                                     skip_group_check=True)
                    nc.tensor.matmul(g_ps[:, gi * B:(gi + 1) * B],
                                     wih1Tb_t[d][:, gi * H:(gi + 1) * H], yb,
                                     start=False, stop=False,
                                     skip_group_check=True)
                    nc.tensor.matmul(g_ps[:, gi * B:(gi + 1) * B],
                                     ctlT66_t[d][:, gi * H:(gi + 1) * H], ct,
                                     start=False, stop=False,
                                     skip_group_check=True)

            def y1loc(c, p):
                sl = sp1(c, p)
                return sl if c[1] == "f" else sl - WARM

            def l1_hprev(c, p):
                if p == 0:
                    return h00[:]
                sl = y1loc(c, p - 1)
                return y1[c][:, sl * B:(sl + 1) * B]

            def l1_yout(c, p):
                sl = y1loc(c, p)
                return y1[c][:, sl * B:(sl + 1) * B]

            run_phase(SPAN1, l1_inproj, {c: whhT1_t[c[1]] for c in CHAINS},
                      l1_hprev, l1_yout)

            # ---------------- classifier (window slots only) ----------------
            # window token u of sub-window w: spatial slot WARM+u ->
            #   y1f local slot WARM+u; y1b local slot u.  Both contiguous.
            NTOK = SW * B                       # tokens*batch per sub-window = 2048
            CH = 512
            h1 = {}
            for w in range(S):
                cf, cb = (w, "f"), (w, "b")
                h1[w] = [clssb.tile([H, NTOK], BF16, name=f"h1a{w}", tag=f"h1a{w}", bufs=1),
                         clssb.tile([H, NTOK], BF16, name=f"h1b{w}", tag=f"h1b{w}", bufs=1)]
                for c0 in range(0, NTOK, CH):
                    xs0 = (WARM + SW * w) * B + c0   # xaug col of token chunk
                    for m in range(2):
                        p = gpool[w].tile([H, CH], F32, name="pc", tag=f"g{w}")
                        nc.tensor.matmul(p[:], w1Ta_t[:, m * H:(m + 1) * H],
                                         y1[cf][:, WARM * B + c0:WARM * B + c0 + CH],
                                         start=True, stop=False)
                        nc.tensor.matmul(p[:], w1Tb_t[:, m * H:(m + 1) * H],
                                         y1[cb][:, c0:c0 + CH],
                                         start=False, stop=False)
                        nc.tensor.matmul(p[:], b1rowP_t[:, m * H:(m + 1) * H],
                                         xaug_t[:, xs0:xs0 + CH],
                                         start=False, stop=True)
                        nc.scalar.activation(h1[w][m][:, c0:c0 + CH], p[:], ACT_RELU)

            # final GEMM transposed: out[tok, d] (tokens on partitions);
            # batch tanh over 4 chunks of 128 token-rows (one PSUM tile).
            for w in range(S):
                for blk in range(0, NTOK, 4 * H):
                    p = gpool[w].tile([H, 4 * D], F32, name="po", tag=f"g{w}")
                    for k in range(4):
                        c0 = blk + k * H
                        nc.tensor.matmul(p[:, k * D:(k + 1) * D],
                                         h1[w][0][:, c0:c0 + H], w2Ta_t[:],
                                         start=(k == 0), stop=False,
                                         skip_group_check=True)
                        nc.tensor.matmul(p[:, k * D:(k + 1) * D],
                                         h1[w][1][:, c0:c0 + H], w2Tb_t[:],
                                         start=False, stop=False,
                                         skip_group_check=True)
                        nc.tensor.matmul(p[:, k * D:(k + 1) * D],
                                         xaug_t[:, (WARM + SW * w) * B + c0:(WARM + SW * w) * B + c0 + H],
                                         b2rowP_t[:],
                                         start=False, stop=(k == 3),
                                         skip_group_check=True)
                    o_t = clssb.tile([H, 4 * D], F32, name="ot", tag="ot")
                    nc.scalar.activation(o_t[:], p[:], ACT_TANH)
                    nc.sync.dma_start(
                        out=out[w * NTOK + blk:w * NTOK + blk + 4 * H, :].rearrange(
                            "(k p) d -> p k d", k=4),
                        in_=o_t[:].rearrange("p (k d) -> p k d", k=4))

    nc.compile()
    return nc


# ======================= host side =======================

def _prep_weights(inp):
    """Returns dict of np arrays shared by all cores (bf16).

    Gate row-blocks reordered from reference [i,f,g,o] to device [o,i,f,g];
    i,f,o rows scaled 0.5 (one-tanh trick)."""
    H_ = H
    sr = np.full((4 * H_, 1), 0.5, np.float32)
    sr[2 * H_:3 * H_] = 1.0

    def reorder(a):           # rows [i,f,g,o] -> [o,i,f,g]
        return np.concatenate([a[3 * H_:], a[:H_], a[H_:2 * H_], a[2 * H_:3 * H_]], 0)

    w = {}
    for d, tag in (("f", "0"), ("b", "1")):
        Wih, Whh = inp[f"Wih0{tag}"], inp[f"Whh0{tag}"]
        bias = inp[f"bih0{tag}"] + inp[f"bhh0{tag}"]
        w[f"wihT0{d}"] = reorder(np.concatenate([Wih * sr, (bias[:, None] * sr)], 1)).T.astype(bf16)
        w[f"whhT0{d}"] = reorder(Whh * sr * 0.5).T.astype(bf16)
        Wih1, Whh1 = inp[f"Wih1{tag}"], inp[f"Whh1{tag}"]
        bias1 = reorder((inp[f"bih1{tag}"] + inp[f"bhh1{tag}"])[:, None] * sr).T
        w[f"whhT1{d}"] = reorder(Whh1 * sr * 0.5).T.astype(bf16)
        w[f"wih1Ta{d}"] = reorder(Wih1[:, :H] * sr * 0.5).T.astype(bf16)
        w[f"wih1Tb{d}"] = reorder(Wih1[:, H:] * sr * 0.5).T.astype(bf16)
        padkill = np.zeros((1, 4 * H), np.float32)
        padkill[0, H:2 * H] = PADKILL      # i-gate block (device order [o,i,f,g])
        ct = np.zeros((D + 2, 4 * H), np.float32)
        ct[D] = bias1
        ct[D + 1] = padkill
        w[f"ctlT66{d}"] = ct.astype(bf16)
    w["w1Ta"] = (0.5 * inp["W1"][:, :H]).T.astype(bf16)
    w["w1Tb"] = (0.5 * inp["W1"][:, H:]).T.astype(bf16)
    b1p = np.zeros((D + 2, 2 * H), np.float32)
    b1p[D] = inp["b1"]
    w["b1rowP"] = b1p.astype(bf16)
    w["w2Ta"] = inp["W2"][:, :H].T.astype(bf16)
    w["w2Tb"] = inp["W2"][:, H:].T.astype(bf16)
    b2p = np.zeros((D + 2, D), np.float32)
    b2p[D] = inp["b2"]
    w["b2rowP"] = b2p.astype(bf16)
    return w


def _per_core_inputs(x, q):
    """x: [B, T, D] f32.  Builds xaug [66, SPANX*B]: x rows, valid row, pad row."""
    t0 = WIN * q - WARM
    xaug = np.zeros((D + 2, SPANX * B), np.float32)
    for s in range(SPANX):
        t = t0 + s
        sl = slice(s * B, (s + 1) * B)
        if 0 <= t < T:
            xaug[:D, sl] = x[:, t, :].T
            xaug[D, sl] = 1.0
        else:
            xaug[D + 1, sl] = 1.0
    return xaug.astype(bf16)


def _get_program():
    if "nc" not in _CACHE:
        _CACHE["nc"] = _build_program()
    return _CACHE["nc"]


def _run(inputs, trace=False):
    inp = {k: np.asarray(v) for k, v in inputs.items()}
    nc = _get_program()
    w = _prep_weights(inp)
    x = inp["x"].astype(np.float32)
    in_maps = []
    for q in range(NC):
        m = dict(w)
        m["xaug"] = _per_core_inputs(x, q)
        in_maps.append(m)
    res = run_bass_kernel_spmd(nc, in_maps, list(range(NC)), trace=trace)
    outp = np.zeros((B, T, D), np.float32)
    for q in range(NC):
        o = res.results[q]["out"].reshape(WIN, B, D)        # [tok, b, d]
        outp[:, WIN * q:WIN * (q + 1), :] = o.transpose(1, 0, 2)
    return outp, res


def kernel(**inputs):
    out, _ = _run(inputs, trace=False)
    return out
